# revision 42
# baseline (speedup 1.0000x reference)
"""Trainium2 Bass kernel for the parameterized-quantum-circuit policy network.

Math: the circuit is psi = V5 E4 V4 ... E0 V0 e0 where V_l are x-independent
1024x1024 unitaries (single-qubit rotations + CZ ring, all built from theta)
and E_l(x) = tensor-prod of Rx(lam*x). Using Rx = H Rz H (H = Hadamard^{ox10}),
E_l = H D_l(x) H with D_l diagonal. Folding the H's into the V's:

    psi = W5 D4 W4 D3 W3 D2 W2 D1 W1 (D0 * psi1)

with W_l = H V_l H (l=1..4), W5 = V5 H, psi1 = first column of H V0, and
D_l[b,k] = exp(-i * phi), phi = sum_q (1-2 bits[k,q]) * lam[l,q] * x[b,q] / 2.

Device work per core (batch-sharded 2048 -> 8 x 256, state [1024, 256] with
dim on partitions): per layer one complex 1024x1024 matmul done as Karatsuba
(3 real matmuls: t1=C@a, t2=D@b, t3=(C+D)@(a+b); re=t1-t2, im=t3-t1-t2) with
bf16 weights/state, diagonal phase multiplies (phi via K=10 matmul, sin/cos
on ACT, no range reduction - |phi| <= ~13 is within the HW Sin table range),
readout sum(|psi|^2 * Zsign) via M=1 reduce matmuls, sigmoid for the 2-way
softmax. All theta/lam/w-derived tables are host-precomputed; all x-dependent
compute runs on device. Elementwise work is spread across DVE/Pool/ACT.
"""

import sys

sys.path.insert(0, "/opt/trn_rl_repo")

import numpy as np
import concourse.bass as bass
import concourse.mybir as mybir
import concourse.tile as tile
from concourse.bass_utils import run_bass_kernel_spmd

F32 = mybir.dt.float32
F32R = mybir.dt.float32r
BF16 = mybir.dt.bfloat16
AF = mybir.ActivationFunctionType
ALU = mybir.AluOpType

NQ = 10
DIM = 1024
L = 5
B = 2048
NC = 8
BC = B // NC  # 256 batch per core
KT = DIM // 128  # 8 k tiles
BETA = 1.0

PI = float(np.pi)
MAGIC = float(1.5 * 2**23)
TWOPI = float(2.0 * np.pi)


# ---------------------------------------------------------------- host math
_bits = (np.arange(DIM)[:, None] >> (NQ - 1 - np.arange(NQ))) & 1
_SIGNS = (1.0 - 2.0 * _bits).astype(np.float64)
_cz = np.ones(DIM)
for _i in range(NQ):
    _cz *= 1.0 - 2.0 * (_bits[:, _i] * _bits[:, (_i + 1) % NQ])
_ZSIGN = (1.0 - 2.0 * (_bits.sum(1) % 2)).astype(np.float64)


def _rx(t):
    c, s = np.cos(0.5 * t), np.sin(0.5 * t)
    return np.array([[c, -1j * s], [-1j * s, c]])


def _ry(t):
    c, s = np.cos(0.5 * t), np.sin(0.5 * t)
    return np.array([[c, -s], [s, c]])


def _rz(t):
    e = np.exp(-0.5j * t)
    return np.array([[e, 0.0], [0.0, np.conj(e)]])


def _build_weights(theta, lam):
    th = np.asarray(theta, np.float64).reshape(L + 1, NQ, 3)
    lm = np.asarray(lam, np.float64).reshape(L, NQ)
    H1 = np.array([[1.0, 1.0], [1.0, -1.0]]) / np.sqrt(2.0)
    H = np.array([[1.0]])
    for _ in range(NQ):
        H = np.kron(H, H1)
    V = []
    for l in range(L + 1):
        U = np.array([[1.0]], dtype=np.complex128)
        for q in range(NQ):
            U = np.kron(U, _rz(th[l, q, 2]) @ _ry(th[l, q, 1]) @ _rx(th[l, q, 0]))
        V.append(_cz[:, None] * U)
    psi1 = (H @ V[0])[:, 0]
    W = [H @ V[l] @ H for l in range(1, L)] + [V[L] @ H]
    A = np.empty((L, NQ, DIM))
    for l in range(L):
        A[l] = (_SIGNS * (lm[l] / 2.0)).T
    return W, psi1, A


# ---------------------------------------------------------------- device IR
def _legalize_single_wait(nc):
    """This walrus build accepts only one sync-wait per instruction: hoist
    extra waits onto injected single-wait EventSemaphore carriers."""
    n_fix = 0
    for f in nc.m.functions:
        for bb in f.blocks:
            insts = bb.instructions
            new = []
            for ins in insts:
                si = ins.sync_info
                if si is not None and len(si.on_wait) > 1:
                    for w in si.on_wait[:-1]:
                        n_fix += 1
                        ev = mybir.InstEventSemaphore(
                            name=f"waitfix_{ins.name}_{n_fix}", ins=[], outs=[]
                        )
                        ev.engine = ins.engine
                        ev.sync_info = mybir.SyncInfo(on_wait=[w], on_update=[])
                        new.append(ev)
                    ins.sync_info = mybir.SyncInfo(
                        on_wait=[si.on_wait[-1]], on_update=si.on_update
                    )
                new.append(ins)
            insts[:] = new
    return n_fix


def _build_nc(cfg="kbf", debug=False, repeat=1, internal_weights=False):
    """cfg: 'kbf' = Karatsuba bf16 no-range-reduction,
    'kbfh' = kbf + half readout (ez = 2*||V+ psi||^2 - ||psi||^2, V+ = rows
    of W5 with zsign=+1 - halves the last layer's matmuls),
    '*_rr' = with rint range reduction on phases."""
    range_reduce = cfg.endswith("_rr")
    half_readout = cfg.startswith("kbfh")
    nc = bass.Bass()
    wdt = BF16  # weight / state dtype
    wkind = "Internal" if internal_weights else "ExternalInput"

    # K=11: row 10 of xt is constant 1.0, row 10 of at holds -delta/2pi for
    # l=0 (phase of psi1, folded into the phase matmul) and 0 for l>0
    xt_d = nc.dram_tensor("xt", [NQ + 1, BC], F32R, kind="ExternalInput")
    at_d = nc.dram_tensor("at", [NQ + 1, L, DIM], F32R, kind="ExternalInput")
    zs_d = nc.dram_tensor("zs", [128, KT], wdt, kind="ExternalInput")
    wsc_d = nc.dram_tensor("wsc", [1, 1], F32, kind="ExternalInput")
    wall_d = {}
    for l in range(1, L + 1):
        # [C | D | S=C+D] x 2 mi, each 128 cols: [KT, npass, 128, 768]
        npass = 2 if (half_readout and l == L) else 4
        wall_d[l] = nc.dram_tensor(f"wall{l}", [KT, npass, 128, 768], wdt, kind=wkind)
    probs_d = nc.dram_tensor("probs", [2, BC], F32, kind="ExternalOutput")
    if debug:
        dbga_d = nc.dram_tensor("dbga", [L + 1, 128, KT, BC], F32, kind="ExternalOutput")
        dbgb_d = nc.dram_tensor("dbgb", [L + 1, 128, KT, BC], F32, kind="ExternalOutput")

    with tile.TileContext(nc) as tc:
        with (
            tc.tile_pool(name="consts", bufs=1) as cpool,
            tc.tile_pool(name="state", bufs=2) as spool,
            tc.tile_pool(name="wts", bufs=4) as wpool,
            tc.tile_pool(name="trig", bufs=1) as tpool,
            tc.tile_pool(name="scr", bufs=4) as upool,
            tc.tile_pool(name="outp", bufs=1) as opool,
            tc.tile_pool(name="psum", bufs=1, space="PSUM") as ppool,
        ):
            # ---- constants
            xt_t = cpool.tile([NQ + 1, BC], F32R)
            nc.sync.dma_start(xt_t[:], xt_d[:])
            at_t = cpool.tile([NQ + 1, L, DIM], F32R)
            for l in range(L):
                eng = (nc.sync, nc.gpsimd, nc.scalar, nc.gpsimd, nc.sync)[l]
                eng.dma_start(at_t[:, l, :], at_d[:, l, :])
            zs_t = cpool.tile([128, KT], wdt)
            nc.sync.dma_start(zs_t[:], zs_d[:])
            wsc_t = cpool.tile([1, 1], F32)
            nc.sync.dma_start(wsc_t[:], wsc_d[:])
            zbias = cpool.tile([128, 1], F32)
            nc.vector.memset(zbias[:], 0.0)
            ones_t = cpool.tile([128, 1], BF16)
            nc.vector.memset(ones_t[:], 1.0)
            hpi = cpool.tile([128, 1], F32)
            nc.vector.memset(hpi[:], PI / 2)
            zb1 = cpool.tile([1, 1], F32)
            nc.vector.memset(zb1[:], 0.0)

            def compute_phase(l, ct_t, st_t):
                """ct/st [128, KT, BC] <- cos/sin(phi) per k-tile; phi comes
                straight out of the PE (A tables carry the /2 factor)."""
                for t in range(KT):
                    phi_p = ppool.tile(
                        [128, BC], F32, tag=f"scratch{t % 2}", name="phi_p"
                    )
                    nc.tensor.matmul(
                        phi_p[:],
                        at_t[:, l, 128 * t : 128 * (t + 1)],
                        xt_t[:],
                        start=True,
                        stop=True,
                    )
                    if not range_reduce:
                        # HW Sin is only accurate for |arg| <= ~4.18, so
                        # range-reduce once: fr = phi' - rint(phi') in
                        # [-0.5,0.5] (phi' = phi/2pi from the PE). Then
                        # sin = Sin(2pi*fr); cos via half-angle:
                        # g = Sin(pi*fr + pi/2) = cos(pi*fr), cos = 2g^2-1.
                        n1 = upool.tile([128, BC], F32, tag="rn", name="n1")
                        nc.vector.tensor_scalar(
                            n1[:], phi_p[:], MAGIC, -MAGIC, ALU.add, ALU.add
                        )
                        fr = upool.tile([128, BC], F32, tag="rf", name="fr")
                        nc.vector.scalar_tensor_tensor(
                            fr[:], n1[:], -1.0, phi_p[:], ALU.mult, ALU.add
                        )
                        nc.scalar.activation(
                            st_t[:, t, :], fr[:], AF.Sin, bias=zbias[:], scale=TWOPI
                        )
                        g = upool.tile([128, BC], F32, tag="rg", name="g")
                        nc.scalar.activation(
                            g[:], fr[:], AF.Sin, bias=hpi[:], scale=PI
                        )
                        gs = upool.tile([128, BC], F32, tag="rh", name="gs")
                        nc.scalar.activation(
                            gs[:], g[:], AF.Square, bias=zbias[:], scale=1.0
                        )
                        nc.vector.tensor_scalar(
                            ct_t[:, t, :], gs[:], 2.0, -1.0, ALU.mult, ALU.add
                        )
                    else:
                        # phi' = phi/2pi from the PE (A tables pre-divided),
                        # rint range reduction, Sin scale=2pi.
                        n1 = upool.tile([128, BC], F32, tag="rn", name="n1")
                        nc.vector.tensor_scalar(
                            n1[:], phi_p[:], MAGIC, -MAGIC, ALU.add, ALU.add
                        )
                        fr = upool.tile([128, BC], F32, tag="rf", name="fr")
                        nc.vector.scalar_tensor_tensor(
                            fr[:], n1[:], -1.0, phi_p[:], ALU.mult, ALU.add
                        )
                        nc.scalar.activation(
                            st_t[:, t, :], fr[:], AF.Sin, bias=zbias[:], scale=TWOPI
                        )
                        n2 = upool.tile([128, BC], F32, tag="rn", name="n2")
                        nc.vector.tensor_scalar(
                            n2[:], phi_p[:], 0.25, MAGIC, ALU.add, ALU.add
                        )
                        nc.vector.tensor_scalar_add(n2[:], n2[:], -MAGIC)
                        fr2 = upool.tile([128, BC], F32, tag="rf", name="fr2")
                        nc.vector.scalar_tensor_tensor(
                            fr2[:], n2[:], -1.0, phi_p[:], ALU.mult, ALU.add
                        )
                        nc.scalar.activation(
                            ct_t[:, t, :], fr2[:], AF.Sin, bias=hpi[:], scale=TWOPI
                        )

            def emit_round(dump_debug):
                # phases computed lazily: layer l's tables just before its
                # matmuls so the ACT sins overlap the previous layer's PE work
                cts, sts = {}, {}

                def phase_tables(l):
                    cts[l] = tpool.tile([128, KT, BC], F32, tag=f"ct{l}", name=f"ct{l}")
                    sts[l] = tpool.tile([128, KT, BC], F32, tag=f"st{l}", name=f"st{l}")
                    compute_phase(l, cts[l], sts[l])

                # ---- init: state = D_0 * psi1 (a,b bf16; s = a+b for Karatsuba)
                phase_tables(0)
                ct0, st0 = cts[0], sts[0]
                a_t = spool.tile([128, KT, BC], wdt, tag="sa", name="a0")
                b_t = spool.tile([128, KT, BC], wdt, tag="sb", name="b0")
                s_t = spool.tile([128, KT, BC], wdt, tag="ss", name="s0")
                for t in range(KT):
                    u1 = upool.tile([128, BC], F32, tag="u1", name="iu1")
                    nc.vector.tensor_scalar_mul(u1[:], ct0[:, t, :], psire_t[:, t : t + 1])
                    nc.vector.scalar_tensor_tensor(
                        a_t[:, t, :], st0[:, t, :], psiim_t[:, t : t + 1], u1[:],
                        ALU.mult, ALU.add,
                    )
                    u2 = upool.tile([128, BC], F32, tag="u2", name="iu2")
                    nc.vector.tensor_scalar_mul(u2[:], st0[:, t, :], psire_t[:, t : t + 1])
                    nc.vector.scalar_tensor_tensor(
                        b_t[:, t, :], ct0[:, t, :], psiim_t[:, t : t + 1], u2[:],
                        ALU.mult, ALU.subtract,
                    )
                    nc.gpsimd.tensor_add(s_t[:, t, :], a_t[:, t, :], b_t[:, t, :])
                if dump_debug:
                    nc.sync.dma_start(dbga_d[0], a_t[:])
                    nc.sync.dma_start(dbgb_d[0], b_t[:])

                # ---- layers
                for l in range(1, L + 1):
                    if l < L:
                        phase_tables(l)
                        ctl, stl = cts[l], sts[l]
                    last_half = half_readout and l == L
                    if last_half:
                        # ||psi||^2 of the layer-4 output, overlapping layer 5
                        nrm_p = ppool.tile([1, BC], F32, tag="scratch1", name="nrm")
                        for t in range(KT):
                            q1 = upool.tile([128, BC], wdt, tag="u1", name="nq1")
                            q2 = upool.tile([128, BC], wdt, tag="u2", name="nq2")
                            nc.gpsimd.tensor_mul(q1[:], a_t[:, t, :], a_t[:, t, :])
                            nc.gpsimd.tensor_mul(q2[:], b_t[:, t, :], b_t[:, t, :])
                            nc.tensor.matmul(
                                nrm_p[:], ones_t[:], q1[:],
                                start=(t == 0), stop=False, skip_group_check=True,
                            )
                            nc.tensor.matmul(
                                nrm_p[:], ones_t[:], q2[:],
                                start=False, stop=(t == KT - 1),
                                skip_group_check=True,
                            )
                    nmt = 4 if last_half else KT
                    a2_t = spool.tile([128, nmt, BC], wdt, tag="sa", name="a2")
                    b2_t = spool.tile([128, nmt, BC], wdt, tag="sb", name="b2")
                    s2_t = spool.tile([128, KT, BC], wdt, tag="ss", name="s2")
                    rhs_a, rhs_b, rhs_s = a_t, b_t, s_t
                    for pass_ in range(nmt // 2):
                        # one PSUM bank per accumulation group (t1,t2,t3 x 2 mi)
                        banks = {
                            p: [
                                ppool.tile([128, BC], F32, tag=f"m{p}{mi}", name=f"m{p}{mi}")
                                for mi in range(2)
                            ]
                            for p in "cds"
                        }
                        for k in range(KT):
                            wt = wpool.tile(
                                [128, 768], wdt, tag="wall", name="wallt", bufs=12
                            )
                            dma_eng = (
                                nc.sync, nc.sync, nc.gpsimd, nc.sync,
                                nc.scalar, nc.sync, nc.sync, nc.sync,
                            )[k % 8]
                            dma_eng.dma_start(wt[:], wall_d[l][k, pass_])
                            # issue order c, s, d: banks drain in that order
                            # downstream, minimizing next-pass WAR stalls
                            for mi in range(2):
                                off = 384 * mi
                                wc = wt[:, off : off + 128]
                                wd = wt[:, off + 128 : off + 256]
                                ws = wt[:, off + 256 : off + 384]
                                nc.tensor.matmul(
                                    banks["c"][mi][:], wc, rhs_a[:, k, :],
                                    start=(k == 0), stop=(k == KT - 1),
                                    skip_group_check=True,
                                )
                                nc.tensor.matmul(
                                    banks["s"][mi][:], ws, rhs_s[:, k, :],
                                    start=(k == 0), stop=(k == KT - 1),
                                    skip_group_check=True,
                                )
                                nc.tensor.matmul(
                                    banks["d"][mi][:], wd, rhs_b[:, k, :],
                                    start=(k == 0), stop=(k == KT - 1),
                                    skip_group_check=True,
                                )
                        for mi in range(2):
                            mg = 2 * pass_ + mi
                            t1 = banks["c"][mi][:]
                            t2 = banks["d"][mi][:]
                            t3 = banks["s"][mi][:]
                            # re = t1 - t2, im = t3 - t1 - t2 (one PSUM operand
                            # per op: stage t1 into SBUF via ACT copy)
                            s1 = upool.tile([128, BC], F32, tag="u3", name="s1")
                            nc.scalar.copy(s1[:], t1)
                            if l < L:
                                # fuse the diag rotate: consume re/im immediately
                                re_t = upool.tile([128, BC], F32, tag="u4", name="re")
                                nc.vector.tensor_sub(re_t[:], s1[:], t2)
                                w_t = upool.tile([128, BC], F32, tag="u5", name="w")
                                nc.vector.tensor_sub(w_t[:], t3, s1[:])
                                im_t = upool.tile([128, BC], F32, tag="u6", name="im")
                                nc.vector.tensor_sub(im_t[:], w_t[:], t2)
                                # a' = c*re + s*im ; b' = c*im - s*re
                                # (Pool reads SBUF only - PSUM is off-limits)
                                u1 = upool.tile([128, BC], F32, tag="u1", name="du1")
                                u2 = upool.tile([128, BC], F32, tag="u2", name="du2")
                                nc.gpsimd.tensor_mul(u1[:], ctl[:, mg, :], re_t[:])
                                nc.gpsimd.tensor_mul(u2[:], stl[:, mg, :], im_t[:])
                                nc.gpsimd.tensor_add(a2_t[:, mg, :], u1[:], u2[:])
                                u3 = upool.tile([128, BC], F32, tag="u7", name="du3")
                                u4 = upool.tile([128, BC], F32, tag="u8", name="du4")
                                nc.gpsimd.tensor_mul(u3[:], ctl[:, mg, :], im_t[:])
                                nc.gpsimd.tensor_mul(u4[:], stl[:, mg, :], re_t[:])
                                nc.gpsimd.tensor_sub(b2_t[:, mg, :], u3[:], u4[:])
                                nc.gpsimd.tensor_add(
                                    s2_t[:, mg, :], a2_t[:, mg, :], b2_t[:, mg, :]
                                )
                            else:
                                nc.vector.tensor_sub(a2_t[:, mg, :], s1[:], t2)
                                w_t = upool.tile([128, BC], F32, tag="u5", name="w")
                                nc.vector.tensor_sub(w_t[:], t3, s1[:])
                                nc.vector.tensor_sub(b2_t[:, mg, :], w_t[:], t2)
                    if dump_debug and not last_half:
                        nc.sync.dma_start(dbga_d[l], a2_t[:])
                        nc.sync.dma_start(dbgb_d[l], b2_t[:])
                    a_t, b_t, s_t = a2_t, b2_t, s2_t

                # ---- readout
                ez_p = ppool.tile([1, BC], F32, tag="scratch0", name="ez")
                nrd = 4 if half_readout else KT
                for t in range(nrd):
                    sq1 = upool.tile([128, BC], wdt, tag="u1", name="sq1")
                    sq2 = upool.tile([128, BC], wdt, tag="u2", name="sq2")
                    nc.gpsimd.tensor_mul(sq1[:], a_t[:, t, :], a_t[:, t, :])
                    nc.gpsimd.tensor_mul(sq2[:], b_t[:, t, :], b_t[:, t, :])
                    lhs1 = ones_t[:] if half_readout else zs_t[:, t : t + 1]
                    nc.tensor.matmul(
                        ez_p[:], lhs1, sq1[:],
                        start=(t == 0), stop=False, skip_group_check=True,
                    )
                    nc.tensor.matmul(
                        ez_p[:], lhs1, sq2[:],
                        start=False, stop=(t == nrd - 1), skip_group_check=True,
                    )
                if half_readout:
                    # ez = 2*||V+ psi||^2 - ||psi||^2
                    nrm_s = opool.tile([1, BC], F32, tag="nrm_s", name="nrm_s")
                    nc.scalar.copy(nrm_s[:], nrm_p[:])
                    ezf = opool.tile([1, BC], F32, tag="ezf", name="ezf")
                    nc.vector.scalar_tensor_tensor(
                        ezf[:], ez_p[:], 2.0, nrm_s[:], ALU.mult, ALU.subtract
                    )
                    ez_in = ezf[:]
                else:
                    ez_in = ez_p[:]
                p0 = opool.tile([1, BC], F32, tag="p0", name="p0")
                nc.scalar.activation(
                    p0[:], ez_in, AF.Sigmoid, bias=zb1[:], scale=wsc_t[:, :]
                )
                p1 = opool.tile([1, BC], F32, tag="p1", name="p1")
                nc.vector.tensor_scalar(p1[:], p0[:], -1.0, 1.0, ALU.mult, ALU.add)
                nc.sync.dma_start(probs_d[0:1, :], p0[:])
                nc.sync.dma_start(probs_d[1:2, :], p1[:])

            for _rep in range(repeat):
                emit_round(debug and _rep == 0)

    nc.finalize()
    _legalize_single_wait(nc)
    return nc


_NC_CACHE = {}


def _get_nc(cfg="kbf", debug=False, repeat=1, internal_weights=False):
    key = (cfg, bool(debug), int(repeat), bool(internal_weights))
    if key not in _NC_CACHE:
        _NC_CACHE[key] = _build_nc(
            cfg=key[0], debug=key[1], repeat=key[2], internal_weights=key[3]
        )
    return _NC_CACHE[key]


def _make_in_maps(x, theta, lam, w, cfg="kbf"):
    range_reduce = cfg.endswith("_rr")
    half_readout = cfg.startswith("kbfh")
    W, psi1, A = _build_weights(theta, lam)
    at = A.transpose(1, 0, 2) / (2.0 * np.pi)  # phi' = phi/2pi on device
    from ml_dtypes import bfloat16 as _bf16

    shared = {
        "at": np.ascontiguousarray(at).astype(np.float32),
        "psire": np.ascontiguousarray(psi1.real.reshape(KT, 128).T).astype(np.float32),
        "psiim": np.ascontiguousarray(psi1.imag.reshape(KT, 128).T).astype(np.float32),
        "zs": np.ascontiguousarray(_ZSIGN.reshape(KT, 128).T).astype(_bf16),
        "wsc": np.array([[BETA * (float(w[0, 0]) - float(w[0, 1]))]], np.float32),
    }
    for l in range(1, L + 1):
        Wl = W[l - 1]
        if half_readout and l == L:
            Wl = Wl[_ZSIGN > 0, :]  # V+ [512, 1024]
        WT = Wl.T
        npass = WT.shape[1] // 256

        def _pack(plane):
            # [1024, M] -> [KT, npass, 128part, 256cols(2mi x 128)]
            return plane.reshape(KT, 128, npass, 256).transpose(0, 2, 1, 3)

        c = _pack(WT.real)       # [KT,npass,128,256]
        d = _pack(WT.imag)
        s = c + d
        # regroup to [KT,npass,128, 768] = per mi: [C|D|S] each 128
        blk = np.empty((KT, npass, 128, 768))
        for mi in range(2):
            off = 384 * mi
            sl = slice(128 * mi, 128 * mi + 128)
            blk[..., off : off + 128] = c[..., sl]
            blk[..., off + 128 : off + 256] = d[..., sl]
            blk[..., off + 256 : off + 384] = s[..., sl]
        shared[f"wall{l}"] = np.ascontiguousarray(blk).astype(_bf16)
    x = np.asarray(x, np.float32)
    in_maps = []
    for i in range(NC):
        m = dict(shared)
        m["xt"] = np.ascontiguousarray(x[BC * i : BC * (i + 1)].T).astype(np.float32)
        in_maps.append(m)
    return in_maps


def run(x, theta, lam, w, trace=False, cfg="kbf", debug=False, repeat=1):
    nc = _get_nc(cfg, debug, repeat)
    in_maps = _make_in_maps(x, theta, lam, w, cfg=cfg)
    res = run_bass_kernel_spmd(nc, in_maps, list(range(NC)), trace=trace)
    out = np.empty((B, 2), np.float32)
    for i in range(NC):
        out[BC * i : BC * (i + 1)] = res.results[i]["probs"].T
    return out, res


def kernel(x, theta, lam, w):
    out, _ = run(x, theta, lam, w, trace=False, cfg="kbf")
    return out
